# revision 3
# baseline (speedup 1.0000x reference)
"""NeuralGCDE Trainium2 kernel.

Strategy: data-parallel over batch B=32 across 8 NeuronCores (B_loc=4 per
core, graph supports/weights replicated, zero inter-core communication).
Per core, the RK4 time scan (12 steps x 4 stages) runs fully on-device.

Layouts (per core, tokens tok = b*256+n, 1024 tokens, 2 chunks of 512):
  - "folded" state [128, 512]: partition p = 64*chunk + feature
  - XG [128 (k*64+i), 1024]: graph-conv input (k=0: x, k=1: A@x)
  - adaptive per-node weights factorized through the embedding:
      g_out[ho,tok] = sum_c [WGOUT;WGOUT].T @ (Eg-mask . (W_pool_c.T @ XG))
    with the d-reduction and the output projection folded into one
    accumulating matmul chain; node bias via (b_pool@WGOUT).T @ EGU.
All matmuls are fp32r (full PE speed), operands at partition base 0.
The h-pipeline (f-path) and z-pipeline (g-path) are decoupled so stage
s+1's f-path overlaps stage s's g-path.
"""
import sys
import os
import numpy as np

if "/opt/trn_rl_repo" not in sys.path:
    sys.path.insert(0, "/opt/trn_rl_repo")

B, N, T, CIN, HID, EMB, KCH = 32, 256, 13, 2, 64, 10, 2
NCORES = 8
BLOC = B // NCORES          # 4
TOK = BLOC * N              # 1024
NSTEP = T - 1               # 12
NSTAGE = 3 * NSTEP + 1      # 37 distinct spline-derivative tensors

_KERNEL_CACHE = {}
LAST_RES = None  # BassKernelResults of the most recent run (for test.py)


def _dx_stage_index(t, s):
    """Index into the 37-entry dX table for RK stage s of step t."""
    if s < 3:
        return 3 * t + s
    return 3 * (t + 1) if (t + 1) < NSTEP else 3 * NSTEP


def _build(n_steps=NSTEP):
    import concourse.bacc as bacc
    import concourse.tile as tile
    from concourse import mybir
    from contextlib import ExitStack

    F32 = mybir.dt.float32
    F32R = mybir.dt.float32r
    AF = mybir.ActivationFunctionType
    ALU = mybir.AluOpType

    nc = bacc.Bacc("TRN2", target_bir_lowering=False, debug=False,
                   num_devices=NCORES)

    def din(name, shape, dt=F32R):
        return nc.dram_tensor(name, shape, dt, kind="ExternalInput").ap()

    H0F = din("H0F", [128, 512])
    Z0F = din("Z0F", [128, 512])
    WFIN = din("WFIN", [128, 128])      # block-diag dup of Wf_in
    WFHID = din("WFHID", [128, 128])
    WGIN = din("WGIN", [128, 128])
    WFOUT_A = din("WFOUT_A", [128, 128])  # [Wf_out_perm; 0]
    WFOUT_B = din("WFOUT_B", [128, 128])  # [0; Wf_out_perm]
    WGOUTD = din("WGOUTD", [128, 128])    # [Wg_out_perm; Wg_out_perm]
    BP2 = din("BP2", [10, 128])           # b_pool @ Wg_out_perm
    BFIN2 = din("BFIN2", [128, 1], F32)
    BFHID2 = din("BFHID2", [128, 1], F32)
    BGIN2 = din("BGIN2", [128, 1], F32)
    BFOUT = din("BFOUT", [128, 1], F32)   # i-major permuted
    BGOUT = din("BGOUT", [128, 1], F32)
    AT0 = din("AT0", [128, 256])          # A.T rows 0:128
    AT1 = din("AT1", [128, 256])
    WP = din("WP", [128, 640])            # [k*64+i, d*64+o]
    EGU = din("EGU", [10, 1024])          # Eg[n(tok), d]
    EGT = din("EGT", [5, 128, 1024], F32)  # per-chunk Eg masks
    IDENT = din("IDENT", [64, 64])
    DXB = din("DXB", [NSTAGE, 128, 1024], F32)
    ZOUT = nc.dram_tensor("ZOUT", [NSTEP, 128, 512], F32R,
                          kind="ExternalOutput").ap()

    _ts = bool(os.environ.get("GCDE_TRACESIM"))
    with tile.TileContext(nc, trace_sim=_ts) as tc, ExitStack() as ctx:
        cp = ctx.enter_context(tc.tile_pool(name="const", bufs=1))
        wk = ctx.enter_context(tc.tile_pool(name="work", bufs=1))
        mk = ctx.enter_context(tc.tile_pool(name="mk", bufs=2))
        st = ctx.enter_context(tc.tile_pool(name="state", bufs=2))
        vp = ctx.enter_context(tc.tile_pool(name="vpool", bufs=4))
        dxp = ctx.enter_context(tc.tile_pool(name="dxp", bufs=2))
        ps128 = ctx.enter_context(tc.tile_pool(name="ps128", bufs=2, space="PSUM"))
        pwide = ctx.enter_context(tc.tile_pool(name="pwide", bufs=4, space="PSUM"))
        pgo = ctx.enter_context(tc.tile_pool(name="pgo", bufs=2, space="PSUM"))

        # ---- resident constants ----
        def cload(src, shape, tag, dt=F32R):
            t = cp.tile(shape, dt, tag=tag)
            nc.sync.dma_start(t[:], src)
            return t

        wfin = cload(WFIN, [128, 128], "wfin")
        wfhid = cload(WFHID, [128, 128], "wfhid")
        wgin = cload(WGIN, [128, 128], "wgin")
        wfout_a = cload(WFOUT_A, [128, 128], "wfout_a")
        wfout_b = cload(WFOUT_B, [128, 128], "wfout_b")
        wgoutd = cload(WGOUTD, [128, 128], "wgoutd")
        bp2 = cload(BP2, [10, 128], "bp2")
        bfin2 = cload(BFIN2, [128, 1], "bfin2", F32)
        bfhid2 = cload(BFHID2, [128, 1], "bfhid2", F32)
        bgin2 = cload(BGIN2, [128, 1], "bgin2", F32)
        bfout = cload(BFOUT, [128, 1], "bfout", F32)
        bgout = cload(BGOUT, [128, 1], "bgout", F32)
        at0 = cload(AT0, [128, 256], "at0")
        at1 = cload(AT1, [128, 256], "at1")
        wp = cload(WP, [128, 640], "wp")
        egu = cload(EGU, [10, 1024], "egu")
        ident = cload(IDENT, [64, 64], "ident")
        egt = []
        for c in range(5):
            t = cp.tile([128, 1024], F32, tag=f"egt{c}")
            nc.sync.dma_start(t[:], EGT[c])
            egt.append(t)

        # ---- state ----
        h = st.tile([128, 512], F32R, tag="h")
        z = st.tile([128, 512], F32R, tag="z")
        nc.sync.dma_start(h[:], H0F)
        nc.sync.dma_start(z[:], Z0F)

        def vf(hin, zin, sidx, kh_tag, kz_tag):
            """One vector-field eval -> (kh, kz) folded [128,512] F32."""
            dxb = dxp.tile([128, 1024], F32, tag="dxb")
            nc.sync.dma_start(dxb[:], DXB[sidx])

            # ---- f path (h-pipeline; independent of z) ----
            pf1 = ps128.tile([128, 512], F32, tag="m")
            nc.tensor.matmul(pf1[:], wfin[:], hin[:], start=True, stop=True)
            x1 = wk.tile([128, 512], F32R, tag="x1")
            nc.scalar.activation(x1[:], pf1[:], AF.Relu, bias=bfin2[:], scale=1.0)

            pf2 = ps128.tile([128, 512], F32, tag="m")
            nc.tensor.matmul(pf2[:], wfhid[:], x1[:], start=True, stop=True)
            x2 = wk.tile([128, 512], F32R, tag="x2")
            nc.scalar.activation(x2[:], pf2[:], AF.Relu, bias=bfhid2[:], scale=1.0)

            ffold = []
            for half, wo in ((0, wfout_a), (1, wfout_b)):
                pF = ps128.tile([128, 512], F32, tag="m")
                nc.tensor.matmul(pF[:], wo[:], x2[:], start=True, stop=True)
                Ff = mk.tile([128, 512], F32, tag=f"F{half}")
                nc.scalar.activation(Ff[:], pF[:], AF.Tanh, bias=bfout[:], scale=1.0)
                ffold.append(Ff)

            # kh = sum_i F_i * dX_i  (dxb rows 0:64 = ch0, 64:128 = ch1)
            kh = mk.tile([128, 512], F32, tag=kh_tag)
            ms = []
            for half in range(2):
                cs = slice(half * 512, (half + 1) * 512)
                os_ = slice(half * 64, (half + 1) * 64)
                Ff = ffold[half]
                m0 = mk.tile([64, 512], F32, tag=f"m0{half}")
                m1 = mk.tile([64, 512], F32, tag=f"m1{half}")
                nc.vector.tensor_tensor(m0[:], Ff[0:64, :], dxb[0:64, cs],
                                        ALU.mult)
                nc.gpsimd.tensor_tensor(m1[:], Ff[64:128, :], dxb[64:128, cs],
                                        ALU.mult)
                nc.gpsimd.tensor_tensor(kh[os_, :], m0[:], m1[:], ALU.add)
                ms.append((m0, m1))

            # ---- g path (z-pipeline; the critical chain) ----
            pg1 = ps128.tile([128, 512], F32, tag="m")
            nc.tensor.matmul(pg1[:], wgin[:], zin[:], start=True, stop=True)
            XG = wk.tile([128, 1024], F32R, tag="XG")
            nc.scalar.activation(XG[0:64, 0:512], pg1[0:64, :], AF.Relu,
                                 bias=bgin2[0:64], scale=1.0)
            nc.vector.tensor_scalar(XG[0:64, 512:1024], pg1[64:128, :],
                                    bgin2[64:128], 0.0, ALU.add, ALU.max)

            # transposes: x [64(i), tok] -> xt[mi] [128(m), (b,i)]
            xt = []
            for mi in range(2):
                pT = pwide.tile([128, 256], F32R, tag="w")
                for b in range(BLOC):
                    nc.tensor.transpose(
                        pT[:, b * 64:(b + 1) * 64],
                        XG[0:64, b * 256 + mi * 128: b * 256 + (mi + 1) * 128],
                        ident[:],
                    )
                xts = wk.tile([128, 256], F32R, tag=f"xt{mi}")
                if mi == 0:
                    nc.vector.tensor_copy(xts[:], pT[:])
                else:
                    nc.scalar.copy(xts[:], pT[:])
                xt.append(xts)

            # support matmul: xg1_b[i, n] = sum_m x[b,m,i] * A.T[m,n]
            for pi in range(2):
                pX = pwide.tile([64, 512], F32, tag="w")
                for bb in range(2):
                    b = pi * 2 + bb
                    sl = slice(b * 64, (b + 1) * 64)
                    bs = slice(bb * 256, (bb + 1) * 256)
                    nc.tensor.matmul(pX[:, bs], xt[0][:, sl], at0[:],
                                     start=True, stop=False)
                    nc.tensor.matmul(pX[:, bs], xt[1][:, sl], at1[:],
                                     start=False, stop=True)
                dst = slice(pi * 512, (pi + 1) * 512)
                if pi == 0:
                    nc.scalar.copy(XG[64:128, dst], pX[:])
                else:
                    nc.vector.tensor_copy(XG[64:128, dst], pX[:])

            # U matmuls + Eg mask + fused (d-reduce @ Wg_out) accumulation
            gfold = []
            for c2 in range(2):
                cs = slice(c2 * 512, (c2 + 1) * 512)
                pg = pgo.tile([128, 512], F32, tag="go")
                for c in range(5):
                    pU = pwide.tile([128, 512], F32, tag="w")
                    nc.tensor.matmul(pU[:], wp[:, c * 128:(c + 1) * 128],
                                     XG[:, cs], start=True, stop=True)
                    V = vp.tile([128, 512], F32R, tag="V")
                    if c % 2 == 0:
                        nc.vector.tensor_tensor(V[:], pU[:], egt[c][:, cs],
                                                ALU.mult)
                    else:
                        Uc = vp.tile([128, 512], F32, tag="Uc")
                        nc.scalar.copy(Uc[:], pU[:])
                        nc.gpsimd.tensor_tensor(V[:], Uc[:], egt[c][:, cs],
                                                ALU.mult)
                    nc.tensor.matmul(pg[:], wgoutd[:], V[:],
                                     start=(c == 0), stop=False,
                                     skip_group_check=True)
                nc.tensor.matmul(pg[:], bp2[:], egu[:, cs],
                                 start=False, stop=True, skip_group_check=True)
                Gf = mk.tile([128, 512], F32, tag=f"G{c2}")
                nc.scalar.activation(Gf[:], pg[:], AF.Tanh, bias=bgout[:],
                                     scale=1.0)
                gfold.append(Gf)

            # kz = sum_i (G*F)_i * dX_i = sum_i G_i * m_i
            kz = mk.tile([128, 512], F32, tag=kz_tag)
            for half in range(2):
                os_ = slice(half * 64, (half + 1) * 64)
                Gf = gfold[half]
                m0, m1 = ms[half]
                gi1 = mk.tile([64, 512], F32, tag=f"gi1{half}")
                nc.gpsimd.tensor_copy(gi1[:], Gf[64:128, :])
                n0 = mk.tile([64, 512], F32, tag=f"n0{half}")
                n1 = mk.tile([64, 512], F32, tag=f"n1{half}")
                nc.vector.tensor_tensor(n0[:], Gf[0:64, :], m0[:], ALU.mult)
                nc.gpsimd.tensor_tensor(n1[:], gi1[:], m1[:], ALU.mult)
                nc.vector.tensor_tensor(kz[os_, :], n0[:], n1[:], ALU.add)
            return kh, kz

        third = 1.0 / 3.0
        for t in range(n_steps):
            k1h, k1z = vf(h, z, _dx_stage_index(t, 0), "k1h", "k1z")

            u2h = wk.tile([128, 512], F32R, tag="u2h")
            u2z = wk.tile([128, 512], F32R, tag="u2z")
            nc.vector.scalar_tensor_tensor(u2h[:], k1h[:], third, h[:],
                                           ALU.mult, ALU.add)
            nc.vector.scalar_tensor_tensor(u2z[:], k1z[:], third, z[:],
                                           ALU.mult, ALU.add)
            k2h, k2z = vf(u2h, u2z, _dx_stage_index(t, 1), "k2h", "k2z")

            ah = wk.tile([128, 512], F32, tag="ah")
            az = wk.tile([128, 512], F32, tag="az")
            u3h = wk.tile([128, 512], F32R, tag="u3h")
            u3z = wk.tile([128, 512], F32R, tag="u3z")
            nc.vector.scalar_tensor_tensor(ah[:], k1h[:], -third, k2h[:],
                                           ALU.mult, ALU.add)
            nc.gpsimd.tensor_tensor(u3h[:], ah[:], h[:], ALU.add)
            nc.vector.scalar_tensor_tensor(az[:], k1z[:], -third, k2z[:],
                                           ALU.mult, ALU.add)
            nc.gpsimd.tensor_tensor(u3z[:], az[:], z[:], ALU.add)
            # u4/final partials that need only k1,k2 (off the critical chain)
            bh = wk.tile([128, 512], F32, tag="bh")
            bz = wk.tile([128, 512], F32, tag="bz")
            nc.gpsimd.tensor_tensor(bh[:], k1h[:], k2h[:], ALU.subtract)
            nc.gpsimd.tensor_tensor(bz[:], k1z[:], k2z[:], ALU.subtract)
            k3h, k3z = vf(u3h, u3z, _dx_stage_index(t, 2), "k3h", "k3z")

            u4h = wk.tile([128, 512], F32R, tag="u4h")
            u4z = wk.tile([128, 512], F32R, tag="u4z")
            nc.gpsimd.tensor_tensor(bh[:], bh[:], k3h[:], ALU.add)
            nc.vector.tensor_tensor(u4h[:], bh[:], h[:], ALU.add)
            nc.gpsimd.tensor_tensor(bz[:], bz[:], k3z[:], ALU.add)
            nc.vector.tensor_tensor(u4z[:], bz[:], z[:], ALU.add)
            # final-sum partials needing only k1..k3
            sh = wk.tile([128, 512], F32, tag="sh")
            sz = wk.tile([128, 512], F32, tag="sz")
            nc.gpsimd.tensor_tensor(sh[:], k2h[:], k3h[:], ALU.add)
            nc.vector.scalar_tensor_tensor(sh[:], sh[:], 3.0, k1h[:],
                                           ALU.mult, ALU.add)
            nc.gpsimd.tensor_tensor(sz[:], k2z[:], k3z[:], ALU.add)
            nc.vector.scalar_tensor_tensor(sz[:], sz[:], 3.0, k1z[:],
                                           ALU.mult, ALU.add)
            k4h, k4z = vf(u4h, u4z, _dx_stage_index(t, 3), "k4h", "k4z")

            # y' = y + (k1 + 3k2 + 3k3 + k4)/8
            hn = st.tile([128, 512], F32R, tag="h")
            zn = st.tile([128, 512], F32R, tag="z")
            nc.gpsimd.tensor_tensor(sz[:], sz[:], k4z[:], ALU.add)
            nc.vector.scalar_tensor_tensor(zn[:], sz[:], 0.125, z[:],
                                           ALU.mult, ALU.add)
            nc.gpsimd.tensor_tensor(sh[:], sh[:], k4h[:], ALU.add)
            nc.vector.scalar_tensor_tensor(hn[:], sh[:], 0.125, h[:],
                                           ALU.mult, ALU.add)
            nc.sync.dma_start(ZOUT[t], zn[:])
            h, z = hn, zn

    nc.compile()
    return nc


def _fold(a):
    """[64, 1024] -> folded [128, 512]."""
    return np.concatenate([a[:, 0:512], a[:, 512:1024]], axis=0)


def _prep_shared(inputs):
    f32 = np.float32
    Eg = np.asarray(inputs["Eg"], f32)
    W_pool = np.asarray(inputs["W_pool"], f32)
    b_pool = np.asarray(inputs["b_pool"], f32)

    logits = Eg @ Eg.T
    r = np.maximum(logits, 0.0)
    e = np.exp(r - r.max(axis=1, keepdims=True))
    A = (e / e.sum(axis=1, keepdims=True)).astype(f32)
    AT = np.ascontiguousarray(A.T)

    WP = np.ascontiguousarray(
        np.transpose(W_pool, (1, 2, 0, 3)).reshape(KCH * HID, EMB * HID)
    ).astype(f32)

    n_of_tok = np.tile(np.arange(N), BLOC)
    EGU = np.ascontiguousarray(Eg.T[:, n_of_tok]).astype(f32)  # [10, 1024]
    EGT = np.empty((5, 128, TOK), f32)
    for c in range(5):
        for dd in range(2):
            EGT[c, dd * 64:(dd + 1) * 64, :] = Eg[n_of_tok, 2 * c + dd][None, :]

    # i-major permutation of the (HID, CIN)-reshaped output dims
    perm = np.empty(HID * CIN, np.int64)
    for i in range(CIN):
        for hh in range(HID):
            perm[i * HID + hh] = hh * CIN + i

    def bd(w):
        out = np.zeros((128, 128), f32)
        out[0:64, 0:64] = w
        out[64:128, 64:128] = w
        return out

    def halfpad(w, top):
        out = np.zeros((128, 128), f32)
        if top:
            out[0:64, :] = w
        else:
            out[64:128, :] = w
        return out

    Wf_out_p = np.asarray(inputs["Wf_out"], f32)[:, perm]
    bf_out_p = np.asarray(inputs["bf_out"], f32)[perm]
    Wg_out_p = np.asarray(inputs["Wg_out"], f32)[:, perm]
    bg_out_p = np.asarray(inputs["bg_out"], f32)[perm]

    shared = {
        "WFIN": bd(np.asarray(inputs["Wf_in"], f32)),
        "WFHID": bd(np.asarray(inputs["Wf_hid"], f32)),
        "WGIN": bd(np.asarray(inputs["Wg_in"], f32)),
        "WFOUT_A": halfpad(Wf_out_p, True),
        "WFOUT_B": halfpad(Wf_out_p, False),
        "WGOUTD": np.concatenate([Wg_out_p, Wg_out_p], axis=0),  # [128,128]
        "BP2": (b_pool @ Wg_out_p).astype(f32),                  # [10,128]
        "BFIN2": np.tile(np.asarray(inputs["bf_in"], f32), 2)[:, None],
        "BFHID2": np.tile(np.asarray(inputs["bf_hid"], f32), 2)[:, None],
        "BGIN2": np.tile(np.asarray(inputs["bg_in"], f32), 2)[:, None],
        "BFOUT": bf_out_p[:, None].astype(f32),
        "BGOUT": bg_out_p[:, None].astype(f32),
        "AT0": np.ascontiguousarray(AT[0:128, :]),
        "AT1": np.ascontiguousarray(AT[128:256, :]),
        "WP": WP,
        "EGU": EGU,
        "EGT": EGT,
        "IDENT": np.eye(64, dtype=f32),
    }
    return shared


def _prep_core(inputs, core, n_steps=NSTEP):
    f32 = np.float32
    ca = np.asarray(inputs["coeff_a"], f32)
    cb = np.asarray(inputs["coeff_b"], f32)
    cc = np.asarray(inputs["coeff_two_c"], f32)
    cd = np.asarray(inputs["coeff_three_d"], f32)
    W_h = np.asarray(inputs["W_h"], f32)
    b_h = np.asarray(inputs["b_h"], f32)
    W_z = np.asarray(inputs["W_z"], f32)
    b_z = np.asarray(inputs["b_z"], f32)

    bsl = slice(core * BLOC, (core + 1) * BLOC)
    x0 = ca[bsl, :, 0, :]                       # [4, 256, 2]
    h0 = (x0 @ W_h + b_h).reshape(TOK, HID).T   # [64, 1024]
    z0 = (x0 @ W_z + b_z).reshape(TOK, HID).T

    # 37 stage dX tensors; rows 0:64 = input chan 0 (bcast to 64
    # partitions), rows 64:128 = chan 1 -- i-major, matching F/G rows.
    DXB = np.empty((NSTAGE, 128, TOK), f32)
    maxidx = T - 2
    for si in range(NSTAGE):
        tt, s = si // 3, si % 3
        tval = tt + s / 3.0
        idx = min(int(np.floor(tval + 1e-9)), maxidx)
        frac = f32(tval - idx)
        dx = cb[bsl, :, idx, :] + (cc[bsl, :, idx, :]
                                   + cd[bsl, :, idx, :] * frac) * frac
        dx = dx.reshape(TOK, CIN)
        DXB[si, 0:64, :] = dx[:, 0][None, :]
        DXB[si, 64:128, :] = dx[:, 1][None, :]

    return {
        "H0F": _fold(h0),
        "Z0F": _fold(z0),
        "DXB": DXB,
    }, (x0 @ W_z + b_z)  # z0 unfolded [4, 256, 64] for output t=0


def kernel(**inputs):
    from concourse.bass_utils import run_bass_kernel_spmd

    n_steps = int(os.environ.get("GCDE_NSTEPS", NSTEP))
    key = n_steps
    if key not in _KERNEL_CACHE:
        _KERNEL_CACHE[key] = _build(n_steps)
    nc = _KERNEL_CACHE[key]

    shared = _prep_shared(inputs)
    in_maps = []
    z0_full = np.empty((B, N, HID), np.float32)
    for core in range(NCORES):
        per, z0c = _prep_core(inputs, core, n_steps)
        z0_full[core * BLOC:(core + 1) * BLOC] = z0c
        in_maps.append({**shared, **per})

    res = run_bass_kernel_spmd(nc, in_maps, list(range(NCORES)))
    global LAST_RES
    LAST_RES = res

    out = np.empty((B, N, T, HID), np.float32)
    out[:, :, 0, :] = z0_full
    for core in range(NCORES):
        Z = res.results[core]["ZOUT"][:n_steps]  # [n_steps, 128, 512]
        zt = np.concatenate([Z[:, 0:64, :], Z[:, 64:128, :]], axis=2)
        # zt: [n_steps, 64, 1024] -> [n_steps, 1024, 64] -> [.., 4, 256, 64]
        zt = zt.transpose(0, 2, 1).reshape(n_steps, BLOC, N, HID)
        for t in range(n_steps):
            out[core * BLOC:(core + 1) * BLOC, :, t + 1, :] = zt[t]
        if n_steps < NSTEP:
            out[:, :, n_steps + 1:, :] = 0.0
    return out

